# revision 1
# baseline (speedup 1.0000x reference)
"""Trainium2 Bass kernel for Swin-style windowed attention.

Problem: x[64,196,768] -> qkv proj -> 12-head attention with relative
position bias -> out proj.  Sharded data-parallel over batch: 8 batch
items per NeuronCore across 8 cores.  All matmuls bf16 with fp32 PSUM
accumulation (measured rel err ~4e-3 vs the fp32 reference).

Per-core design (8 batch items, ~210 us on HW):
 - QKV projection: q,k feature-major ([feat, tok] so each head's 64-dim
   slice sits on partitions), v token-major ([tok, head-pair, 128]).
   x/q/k are tiled per 392-token chunk; inputs arrive via per-partition-
   contiguous repacked layouts (big DMA packets), ordered by first use.
 - Attention in the S^T layout, head pairs processed together:
   S^T[j,i] = sum_d k[d,j] q[d,i] (K=64, the pair row-packed into the
   128-deep PE array).  The pair's two S^T tiles live in ONE two-bank
   psum tile [jlen, 2, 512] (one bank per head; two accumulation chains
   in a single bank crash the exec unit), so a single strided ACT exp
   covers both heads.  Softmax runs along partitions (no max-subtract:
   logits are O(1) by construction).
 - Relative position bias applied multiplicatively after exp:
   pt = exp(S^T) * exp(rpb^T), with exp(rpb^T) precomputed on host; the
   multiply runs on the otherwise-idle GpSimd engine for the 128-row
   chunk and on DVE for the 68-row chunk.
 - PV: ONE [128, 2*196] matmul per ktok chunk with lhsT = [v_h0|v_h1]
   and rhs = [P_h0^T P_h1^T]: the diagonal 64x196 blocks are both
   heads' O^T, off-diagonal blocks are discarded -- same streamed
   columns as two matmuls, half the weight loads.  Softmax sums come
   from ones[jlen,64] matmuls (replicating each head's sums across its
   64 rows, col-packed); one fast DVE reciprocal + two base-aligned
   DVE multiplies normalize O^T.
 - Output projection contracts head pairs (K=128) accumulating over 6
   pairs with the two 384-wide halves' chains interleaved (weight loads
   hide under matmuls).  Bias b_eff = proj_bias + proj_weight @ v_bias
   (v_bias commutes through softmax-normalized attention) is added
   during the PSUM->SBUF move; q_bias*scale is folded into the q
   PSUM->SBUF copy (per-partition ACT bias); scale folded into wq on
   host.
"""

import numpy as np
import ml_dtypes

import concourse.bass as bass
import concourse.mybir as mybir
from concourse.bacc import Bacc
from concourse.bass_utils import run_bass_kernel_spmd
from concourse.tile import TileContext

F32 = mybir.dt.float32
BF16 = mybir.dt.bfloat16
AF = mybir.ActivationFunctionType
ALU = mybir.AluOpType

N_CORES = 8
B, NTOK, DIM = 64, 196, 768
H, HD = 12, 64
NHP = H // 2          # head pairs
BPC = B // N_CORES    # batches per core
TPC = BPC * NTOK      # tokens per core (1568)
SCALE = HD ** -0.5
KC = DIM // 128       # contraction chunks for 768 (6)
TOKC = [(0, 128), (128, 68)]   # token chunking of 196
NQCH = 4              # token N-chunks (1568/392); 392 = 2 batches
NQW = TPC // NQCH     # 392

MERGED_S_CHAIN = False  # two-MM single-bank chain crashes TRN2 (exec unit dies)
RPB_ON_GPSIMD = True    # pt = exp(S)*exp(rpb) on GpSimd vs DVE psum add


def build_nc():
    nc = Bacc()

    # x_t2[p, n, kc, w] = x_fm[kc*128+p, n*392+w] -> per-partition contiguous
    # 6*784B runs per n-chunk (big DMA packets)
    x_t = nc.declare_dram_parameter("x_t", [128, NQCH, KC, NQW], BF16, False)
    # w_all[p, w, kc, o] = W_w^T[kc*128+p, o] for w in (q,k,v,p)
    w_all = nc.declare_dram_parameter("w_all", [128, 4, KC, DIM], BF16, False)
    qb = nc.declare_dram_parameter("qb", [128, KC], F32, False)
    beff = nc.declare_dram_parameter("beff", [128, DIM], F32, False)
    # exp(rpb^T), bf16, split by ktok chunk: [j, head, i]
    rpb0 = nc.declare_dram_parameter("rpb0", [128, H, NTOK], BF16, False)
    rpb1 = nc.declare_dram_parameter("rpb1", [68, H, NTOK], BF16, False)
    y = nc.declare_dram_parameter("y", [TPC, DIM], F32, True)

    with TileContext(nc) as tc, \
         tc.tile_pool(name="const", bufs=1) as cpool:
        def ctile(shape, dtype, nm):
            return cpool.tile(shape, dtype, name=nm, tag=nm)

        # ---------------- inputs (DMA ordered by first use) ----------------
        # wq/wk are stored t-major ([t, kc, 128] per partition) and DMA'd in
        # two t-slices so the t=0 accumulation chains start before the full
        # weight tensors land; wv/wp stay kc-major (contiguous 384-col moving
        # slices -- strided moving APs cost ~60ns/matmul on the PE).
        x_n = [ctile([128, KC, NQW], BF16, f"xn{n}") for n in range(NQCH)]
        wqk_t = [ctile([128, KC, KC, 128], BF16, f"wt{w}") for w in range(2)]
        w_t = [None, None] + [ctile([128, KC, DIM], BF16, f"wt{w}")
                              for w in (2, 3)]
        x_sb = {(kc, n): x_n[n][:, kc, :]
                for kc in range(KC) for n in range(NQCH)}
        wq_sb = {(t, kc): wqk_t[0][:, t, kc, :]
                 for t in range(KC) for kc in range(KC)}
        wk_sb = {(t, kc): wqk_t[1][:, t, kc, :]
                 for t in range(KC) for kc in range(KC)}
        wv_sb = [w_t[2][:, kc, :] for kc in range(KC)]
        wp_sb = [w_t[3][:, kc, :] for kc in range(KC)]

        # Inputs split across BOTH hardware DGE queues (SP=sync and
        # Activation=scalar, idle during the load phase): descriptor
        # generation and issue run in parallel, so the first QKV chain's
        # deps (x_n0 + wq t0-slice on sync; wk on scalar) land sooner.
        qb_sb = ctile([128, KC], F32, "qb_sb")
        rpb0_sb = ctile([128, H, NTOK], BF16, "rpb0_sb")
        rpb1_sb = ctile([68, H, NTOK], BF16, "rpb1_sb")
        beff_bc = ctile([128, DIM], F32, "beff_bc")
        nc.sync.dma_start(x_n[0][:], x_t[:, 0])
        nc.scalar.dma_start(wqk_t[1][:, 0:2], w_all[:, 1, 0:2])
        nc.sync.dma_start(wqk_t[0][:, 0:2], w_all[:, 0, 0:2])
        nc.scalar.dma_start(wqk_t[1][:, 2:KC], w_all[:, 1, 2:KC])
        nc.sync.dma_start(wqk_t[0][:, 2:KC], w_all[:, 0, 2:KC])
        nc.scalar.dma_start(qb_sb[:], qb[:])
        nc.scalar.dma_start(rpb0_sb[:], rpb0[:])
        nc.scalar.dma_start(rpb1_sb[:], rpb1[:])
        nc.sync.dma_start(w_t[2][:], w_all[:, 2])
        for n in range(1, NQCH):
            nc.sync.dma_start(x_n[n][:], x_t[:, n])
        nc.scalar.dma_start(w_t[3][:], w_all[:, 3])
        nc.scalar.dma_start(beff_bc[:], beff[:])

        ones_sb = ctile([128, 128], BF16, "ones_sb")
        nc.vector.memset(ones_sb[:], 1.0)

        # ---------------- persistent activations ----------------
        q_sb = {(t, n): ctile([128, NQW], BF16, f"q{t}_{n}")
                for t in range(KC) for n in range(NQCH)}
        k_sb = {(t, n): ctile([128, NQW], BF16, f"k{t}_{n}")
                for t in range(KC) for n in range(NQCH)}
        v_sb = {}
        for b in range(BPC):
            for ci, (toff, tlen) in enumerate(TOKC):
                v_sb[(b, ci)] = ctile([tlen, NHP, 128], BF16, f"v{b}_{ci}")

        # ---------------- phase A: QKV projection ----------------
        with tc.tile_pool(name="qkv_ps", bufs=4, space="PSUM") as pqk:
            for n in range(NQCH):
                for t in range(KC):
                    psq = pqk.tile([128, NQW], F32, tag="qkps")
                    for kc in range(KC):
                        nc.tensor.matmul(
                            psq[:], wq_sb[(t, kc)],
                            x_sb[(kc, n)][:], start=(kc == 0),
                            stop=(kc == KC - 1))
                    nc.scalar.activation(q_sb[(t, n)][:], psq[:],
                                         AF.Identity, bias=qb_sb[:, t:t + 1])
                    psk = pqk.tile([128, NQW], F32, tag="qkps")
                    for kc in range(KC):
                        nc.tensor.matmul(
                            psk[:], wk_sb[(t, kc)],
                            x_sb[(kc, n)][:], start=(kc == 0),
                            stop=(kc == KC - 1))
                    nc.scalar.activation(k_sb[(t, n)][:], psk[:], AF.Copy)
                for b in (2 * n, 2 * n + 1):
                    for ci, (toff, tlen) in enumerate(TOKC):
                        c0 = (b % 2) * NTOK + toff
                        for nh in range(2):
                            psv = pqk.tile([128, 384], F32, tag="vps")
                            for kc in range(KC):
                                nc.tensor.matmul(
                                    psv[:tlen], x_sb[(kc, n)][:, c0:c0 + tlen],
                                    wv_sb[kc][:, nh * 384:(nh + 1) * 384],
                                    start=(kc == 0), stop=(kc == KC - 1))
                            nc.scalar.activation(
                                v_sb[(b, ci)][:, nh * 3:(nh + 1) * 3, :]
                                .rearrange("p a b -> p (a b)"),
                                psv[:tlen], AF.Copy)

        # ---------------- phase B: attention + out projection ----------------
        _ob = 2 if MERGED_S_CHAIN else 1
        with tc.tile_pool(name="s_ps", bufs=1, space="PSUM") as ps_s, \
             tc.tile_pool(name="o_ps", bufs=1, space="PSUM") as ps_o, \
             tc.tile_pool(name="r_ps", bufs=1, space="PSUM") as ps_r, \
             tc.tile_pool(name="proj_ps", bufs=2, space="PSUM") as ps_proj, \
             tc.tile_pool(name="pr_sbuf", bufs=4) as praw_pool, \
             tc.tile_pool(name="p_sbuf", bufs=4) as p_pool, \
             tc.tile_pool(name="r_sbuf", bufs=3) as r_pool, \
             tc.tile_pool(name="o_sbuf", bufs=14) as o_pool, \
             tc.tile_pool(name="y_sbuf", bufs=6) as y_pool:
            o_tiles = {}

            def stage1(b, hp):
                """S^T matmuls + exp + rpbE multiply -> p_tiles dict."""
                n = b // 2
                q0 = (b % 2) * NTOK
                p_tiles = {}
                for ci, (joff, jlen) in enumerate(TOKC):
                    rpb_sb = rpb0_sb if ci == 0 else rpb1_sb
                    jsl = slice(q0 + joff, q0 + joff + jlen)
                    rpb_pair = rpb_sb[:jlen, 2 * hp:2 * hp + 2, :] \
                        .rearrange("p h n -> p (h n)")
                    pt = p_pool.tile([jlen, 2 * NTOK], BF16, tag=f"p{ci}")
                    praw = praw_pool.tile([jlen, 2 * NTOK], BF16,
                                          tag=f"pr{ci}")
                    # [jlen, 2, 512] = one PSUM bank per head half; a single
                    # strided ACT exp then covers both heads in one op.
                    pss = ps_s.tile([jlen, 2, 512], F32, tag=f"s{ci}")
                    for hh in range(2):
                        rows = slice(hh * 64, hh * 64 + 64)
                        nc.tensor.matmul(
                            pss[:, hh, 0:NTOK], k_sb[(hp, n)][rows, jsl],
                            q_sb[(hp, n)][rows, q0:q0 + NTOK],
                            start=True, stop=True)
                    nc.scalar.activation(
                        praw[:].rearrange("p (a b) -> p a b", a=2),
                        pss[:, :, 0:NTOK], AF.Exp)
                    eng = nc.gpsimd if ci == 0 else nc.vector
                    eng.tensor_tensor(pt[:], praw[:], rpb_pair, ALU.mult)
                    p_tiles[ci] = pt
                return p_tiles

            def stage2(b, hp, p_tiles):
                """PV + sum matmuls, reciprocal, normalize -> o_tiles.

                One [128, 392] matmul per ktok chunk computes BOTH heads'
                O^T: lhsT = [v_h0 | v_h1] (128 cols), rhs = [P_h0^T P_h1^T]
                (392 cols).  Diagonal 64x196 blocks are the real outputs;
                off-diagonal blocks are discarded.  Same streamed columns
                as two separate matmuls, half the weight loads.  The sums
                come from one ones[jlen,128] matmul the same way.
                """
                po = ps_o.tile([128, 2 * NTOK], F32, tag="o")
                psr = ps_r.tile([128, NTOK], F32, tag="r")
                for ci, (joff, jlen) in enumerate(TOKC):
                    nc.tensor.matmul(po[:], v_sb[(b, ci)][:, hp, :],
                                     p_tiles[ci][:], start=(ci == 0),
                                     stop=(ci == 1))
                for hh in range(2):
                    cols = slice(hh * 64, hh * 64 + 64)
                    for ci, (joff, jlen) in enumerate(TOKC):
                        nc.tensor.matmul(
                            psr[cols, :], ones_sb[:jlen, 0:64],
                            p_tiles[ci][:, hh * NTOK:(hh + 1) * NTOK],
                            start=(ci == 0), stop=(ci == 1))
                rbc = r_pool.tile([128, NTOK], F32, tag="rbc")
                nc.vector.reciprocal_approx_fast(out=rbc[:], in_=psr[:])
                ot = o_pool.tile([128, NTOK], BF16, tag="o_sb")
                nc.vector.tensor_tensor(ot[0:64, :], po[0:64, 0:NTOK],
                                        rbc[0:64, :], ALU.mult)
                nc.vector.tensor_tensor(ot[64:128, :],
                                        po[64:128, NTOK:2 * NTOK],
                                        rbc[64:128, :], ALU.mult)
                o_tiles[(b, hp)] = ot

            def proj(b):
                for ci, (toff, tlen) in enumerate(TOKC):
                    psy = [ps_proj.tile([128, 384], F32, tag="proj",
                                        name=f"psy{nh}") for nh in range(2)]
                    for hp in range(NHP):
                        for nh in range(2):
                            nc.tensor.matmul(
                                psy[nh][:tlen],
                                o_tiles[(b, hp)][:, toff:toff + tlen],
                                wp_sb[hp][:, nh * 384:(nh + 1) * 384],
                                start=(hp == 0), stop=(hp == NHP - 1))
                    tok0 = b * NTOK + toff
                    for nh in range(2):
                        yt = y_pool.tile([128, 384], F32, tag="y")
                        nc.vector.tensor_tensor(
                            yt[:tlen], psy[nh][:tlen],
                            beff_bc[:tlen, nh * 384:(nh + 1) * 384], ALU.add)
                        # last batch: overlap the two tail store issues by
                        # putting one on the scalar DGE queue (idle by then)
                        dq = nc.scalar if (b == BPC - 1 and nh == 0) \
                            else nc.sync
                        dq.dma_start(
                            y[tok0:tok0 + tlen, nh * 384:(nh + 1) * 384],
                            yt[:tlen])

            # Software-pipelined emission: the S-stage runs STAGGER blocks
            # ahead of the PV-stage so the in-order PE queue never stalls
            # on the exp -> rpbE-multiply chain.
            STAGGER = 0
            blocks = [(b, hp) for b in range(BPC) for hp in range(NHP)]
            pending = {}
            for idx in range(min(STAGGER, len(blocks))):
                pending[idx] = stage1(*blocks[idx])
            for k, (b, hp) in enumerate(blocks):
                nxt = k + STAGGER
                if nxt < len(blocks):
                    pending[nxt] = stage1(*blocks[nxt])
                stage2(b, hp, pending.pop(k))
                if hp == NHP - 1:
                    proj(b)
    nc.finalize()
    return nc


def prep_host(x, qkv_weight, q_bias, v_bias, rpb_table, rel_pos_index,
              proj_weight, proj_bias):
    """Host-side prep: transposes, dtype casts, bias folding, rpb gather."""
    bf16 = ml_dtypes.bfloat16
    x = np.asarray(x, np.float32)
    qkv_weight = np.asarray(qkv_weight, np.float32)
    proj_weight = np.asarray(proj_weight, np.float32)
    q_bias = np.asarray(q_bias, np.float32)
    v_bias = np.asarray(v_bias, np.float32)
    rpb_table = np.asarray(rpb_table, np.float32)
    rel_pos_index = np.asarray(rel_pos_index)
    proj_bias = np.asarray(proj_bias, np.float32)

    # scale folded into q projection weights + bias
    wq = qkv_weight[0:DIM].T * SCALE
    wk = qkv_weight[DIM:2 * DIM].T
    wv = qkv_weight[2 * DIM:3 * DIM].T
    wp = proj_weight.T
    # wq/wk: w_all[p, w, t, kc*128+c] = W^T[kc*128+p, t*128+c] (t-major);
    # wv/wp: w_all[p, w, kc, o] = W^T[kc*128+p, o] (kc-major, as consumed)
    def kc_major(w):
        return w.reshape(KC, 128, DIM).transpose(1, 0, 2)      # [p, kc, o]

    def t_major(w):
        return (w.reshape(KC, 128, KC, 128)                    # [kc, p, t, c]
                .transpose(1, 2, 0, 3).reshape(128, KC, DIM))  # [p, t, kc*c]

    w_all = np.ascontiguousarray(np.stack(
        [t_major(wq), t_major(wk), kc_major(wv), kc_major(wp)],
        axis=1)).astype(bf16)                                  # [p, 4, ., .]
    qb = np.ascontiguousarray((q_bias * SCALE).reshape(KC, 128).T).astype(np.float32)
    beff = np.ascontiguousarray(np.broadcast_to(
        (proj_bias + proj_weight @ v_bias).reshape(1, DIM), (128, DIM))).astype(np.float32)

    rpb_full = rpb_table[rel_pos_index.reshape(-1)].reshape(NTOK, NTOK, H)
    rpbT = np.exp(np.ascontiguousarray(rpb_full.transpose(1, 2, 0)),
                  dtype=np.float32)
    rpb0 = np.ascontiguousarray(rpbT[0:128]).astype(bf16)
    rpb1 = np.ascontiguousarray(rpbT[128:NTOK]).astype(bf16)

    shared = dict(w_all=w_all, qb=qb, beff=beff, rpb0=rpb0, rpb1=rpb1)
    in_maps = []
    for c in range(N_CORES):
        xs = x[c * BPC:(c + 1) * BPC]                       # [8,196,768]
        x_fm = xs.transpose(2, 0, 1).reshape(DIM, TPC)      # [768, 1568]
        # x_t2[p, n, kc, w] = x_fm[kc*128+p, n*392+w]
        x_tc = np.ascontiguousarray(
            x_fm.reshape(KC, 128, NQCH, NQW).transpose(1, 2, 0, 3)
        ).astype(bf16)
        in_maps.append(dict(shared, x_t=x_tc))
    return in_maps


_NC_CACHE = {}


def get_nc():
    if "nc" not in _NC_CACHE:
        _NC_CACHE["nc"] = build_nc()
    return _NC_CACHE["nc"]


def kernel(**inputs):
    nc = get_nc()
    in_maps = prep_host(**inputs)
    res = run_bass_kernel_spmd(nc, in_maps, list(range(N_CORES)))
    outs = [res.results[c]["y"].reshape(BPC, NTOK, DIM) for c in range(N_CORES)]
    return np.concatenate(outs, axis=0).astype(np.float32)



# revision 19
# speedup vs baseline: 1.0313x; 1.0313x over previous
"""Trainium2 Bass kernel for Swin-style windowed attention.

Problem: x[64,196,768] -> qkv proj -> 12-head attention with relative
position bias -> out proj.  Sharded data-parallel over batch: 8 batch
items per NeuronCore across 8 cores.  All matmuls bf16 with fp32 PSUM
accumulation.

Per-core design (8 batch items):
 - QKV projection: k feature-major; q written block-DIAGONAL per head
   pair (q_diag[d, h, i] = q_h[d] if d in h's half else 0, zero-filled
   once at startup) so ONE matmul computes both heads' S^T; v
   token-major with a 64-wide ones block packed before each head's v
   (slots [ones | v_even | ones | v_odd]).
 - Attention in the S^T layout: S^T pair = k_pair^T @ q_diag, a single
   [128, jlen] x [128, 2x196] matmul per ktok chunk -> one psum bank,
   one chain (TRN2 crashes if two accumulation chains land in one bank
   at column-disjoint ranges).  One contiguous ACT exp covers both
   heads; pt = exp(S^T) * exp(rpb^T) (host-precomputed) on GpSimd (ci0)
   / DVE (ci1).
 - PV fused with softmax sums: lhsT = [ones | v_h] so one chain yields
   the softmax sums replicated on partitions 0:64 AND O_h^T on 64:128.
   Sums land at base 0 because reciprocal_approx_fast is broken at
   nonzero partition bases.  Two base-0 reciprocals + two mixed-base
   DVE multiplies normalize O^T.
 - Output projection feature-major over batch PAIRS: psy[128 feats,
   392 toks] accumulating 6 head-pair chunks.  Chains are interleaved
   into the next pair's attention blocks and share the po psum-bank
   rings.  Bias (proj_bias + proj_weight @ v_bias) is a per-partition
   ACT bias during PSUM->SBUF; y is stored feature-major, transposed on
   host.
 - Startup: first QKV chain's deps (wq t01 slice + x n0) issued first,
   split across both DGE queues.
"""

import os
import numpy as np
import ml_dtypes

import concourse.bass as bass
import concourse.mybir as mybir
from concourse.bacc import Bacc
from concourse.bass_utils import run_bass_kernel_spmd
from concourse.tile import TileContext

F32 = mybir.dt.float32
BF16 = mybir.dt.bfloat16
AF = mybir.ActivationFunctionType
ALU = mybir.AluOpType

N_CORES = 8
B, NTOK, DIM = 64, 196, 768
H, HD = 12, 64
NHP = H // 2          # head pairs
BPC = B // N_CORES    # batches per core
TPC = BPC * NTOK      # tokens per core (1568)
SCALE = HD ** -0.5
KC = DIM // 128       # contraction chunks for 768 (6)
TOKC = [(0, 128), (128, 68)]   # token chunking of 196
NQCH = 4              # token N-chunks (1568/392); 392 = 2 batches
NQW = TPC // NQCH     # 392
STAGGER = int(os.environ.get("KV_STAGGER", "3"))
NBLK = BPC * NHP      # 48


def build_nc():
    nc = Bacc()

    x_t = nc.declare_dram_parameter("x_t", [128, NQCH, KC, NQW], BF16, False)
    w_all = nc.declare_dram_parameter("w_all", [128, 4, KC, DIM], BF16, False)
    qb = nc.declare_dram_parameter("qb", [128, KC], F32, False)
    beff = nc.declare_dram_parameter("beff", [128, KC], F32, False)
    rpb0 = nc.declare_dram_parameter("rpb0", [128, H, NTOK], BF16, False)
    rpb1 = nc.declare_dram_parameter("rpb1", [68, H, NTOK], BF16, False)
    # y feature-major: y[t, p, tok] = y^T[t*128+p, tok]
    y = nc.declare_dram_parameter("y", [KC, 128, TPC], F32, True)

    with TileContext(nc) as tc, \
         tc.tile_pool(name="const", bufs=1) as cpool:
        def ctile(shape, dtype, nm):
            return cpool.tile(shape, dtype, name=nm, tag=nm)

        # ---------------- inputs (DMA ordered by first use) ----------------
        x_n = [ctile([128, KC, NQW], BF16, f"xn{n}") for n in range(NQCH)]
        wqk_t = [ctile([128, KC, KC, 128], BF16, f"wt{w}") for w in range(2)]
        w_t = [None, None] + [ctile([128, KC, DIM], BF16, f"wt{w}")
                              for w in (2, 3)]
        x_sb = {(kc, n): x_n[n][:, kc, :]
                for kc in range(KC) for n in range(NQCH)}
        wq_sb = {(t, kc): wqk_t[0][:, t, kc, :]
                 for t in range(KC) for kc in range(KC)}
        wk_sb = {(t, kc): wqk_t[1][:, t, kc, :]
                 for t in range(KC) for kc in range(KC)}
        wv_sb = [w_t[2][:, kc, :] for kc in range(KC)]

        qb_sb = ctile([128, KC], F32, "qb_sb")
        beff_sb = ctile([128, KC], F32, "beff_sb")
        rpb0_sb = ctile([128, H, NTOK], BF16, "rpb0_sb")
        rpb1_sb = ctile([68, H, NTOK], BF16, "rpb1_sb")

        # ---------------- persistent activations ----------------
        # q block-diagonal: [d_full, parity, tok]; zero-filled, diag written
        q_sb = {(t, n): ctile([128, 2, NQW], BF16, f"q{t}_{n}")
                for t in range(KC) for n in range(NQCH)}
        k_sb = {(t, n): ctile([128, NQW], BF16, f"k{t}_{n}")
                for t in range(KC) for n in range(NQCH)}
        # v with ones slots: [tok, b, hp, parity, {ones,v}, 64]
        v_ci = [ctile([tlen, BPC, NHP, 2, 2, 64], BF16, f"v{ci}")
                for ci, (_, tlen) in enumerate(TOKC)]
        for ci in range(2):
            eng = nc.vector if ci == 0 else nc.gpsimd
            eng.memset(v_ci[ci][:, :, :, :, 0, :]
                       .rearrange("p a b c d -> p (a b c) d"), 1.0)
        for i, tile in enumerate(q_sb.values()):
            eng = nc.vector if i % 2 == 0 else nc.gpsimd
            eng.memset(tile[:].rearrange("p a b -> p (a b)"), 0.0)

        # DMA order: first QKV chain deps first, split across both queues.
        nc.sync.dma_start(wqk_t[0][:, 0:2], w_all[:, 0, 0:2])
        nc.scalar.dma_start(wqk_t[1][:, 0:2], w_all[:, 1, 0:2])
        nc.sync.dma_start(x_n[0][:, 0:3], x_t[:, 0, 0:3])
        nc.scalar.dma_start(x_n[0][:, 3:KC], x_t[:, 0, 3:KC])
        nc.scalar.dma_start(qb_sb[:], qb[:])
        nc.sync.dma_start(wqk_t[0][:, 2:KC], w_all[:, 0, 2:KC])
        nc.scalar.dma_start(wqk_t[1][:, 2:KC], w_all[:, 1, 2:KC])
        nc.sync.dma_start(x_n[1][:], x_t[:, 1])
        nc.scalar.dma_start(w_t[2][:], w_all[:, 2])
        nc.sync.dma_start(x_n[2][:], x_t[:, 2])
        nc.scalar.dma_start(rpb0_sb[:], rpb0[:])
        nc.sync.dma_start(x_n[3][:], x_t[:, 3])
        nc.scalar.dma_start(rpb1_sb[:], rpb1[:])
        nc.sync.dma_start(w_t[3][:], w_all[:, 3])
        nc.scalar.dma_start(beff_sb[:], beff[:])

        # ---------------- phase A: QKV projection ----------------
        with tc.tile_pool(name="qkv_ps", bufs=4, space="PSUM") as pqk:
            for n in range(NQCH):
                for t in range(KC):
                    psq = pqk.tile([128, NQW], F32, tag="qkps")
                    for kc in range(KC):
                        nc.tensor.matmul(
                            psq[:], wq_sb[(t, kc)],
                            x_sb[(kc, n)][:], start=(kc == 0),
                            stop=(kc == KC - 1))
                    # diag halves with qb bias folded (DVE; ACT is busy)
                    qd = q_sb[(t, n)]
                    nc.vector.tensor_scalar(
                        qd[0:64, 0, :], psq[0:64, :],
                        qb_sb[0:64, t:t + 1], None, ALU.add)
                    nc.vector.tensor_scalar(
                        qd[64:128, 1, :], psq[64:128, :],
                        qb_sb[64:128, t:t + 1], None, ALU.add)
                    psk = pqk.tile([128, NQW], F32, tag="qkps")
                    for kc in range(KC):
                        nc.tensor.matmul(
                            psk[:], wk_sb[(t, kc)],
                            x_sb[(kc, n)][:], start=(kc == 0),
                            stop=(kc == KC - 1))
                    nc.scalar.activation(k_sb[(t, n)][:], psk[:], AF.Copy)
                for b in (2 * n, 2 * n + 1):
                    for ci, (toff, tlen) in enumerate(TOKC):
                        c0 = (b % 2) * NTOK + toff
                        for nh in range(2):
                            psv = pqk.tile([128, 384], F32, tag="vps")
                            for kc in range(KC):
                                nc.tensor.matmul(
                                    psv[:tlen], x_sb[(kc, n)][:, c0:c0 + tlen],
                                    wv_sb[kc][:, nh * 384:(nh + 1) * 384],
                                    start=(kc == 0), stop=(kc == KC - 1))
                            # psv cols = (pair, parity, 64) -> v slot 1
                            nc.scalar.activation(
                                v_ci[ci][:tlen, b, 3 * nh:3 * nh + 3, :, 1, :],
                                psv[:tlen].rearrange(
                                    "p (a b c) -> p a b c", a=3, b=2),
                                AF.Copy)

        # ---------------- phase B: attention + out projection ----------------
        with tc.tile_pool(name="s_ps", bufs=2, space="PSUM") as ps_s, \
             tc.tile_pool(name="o_ps", bufs=2, space="PSUM") as ps_o, \
             tc.tile_pool(name="pr_sbuf", bufs=STAGGER + 2) as praw_pool, \
             tc.tile_pool(name="p_sbuf", bufs=STAGGER + 2) as p_pool, \
             tc.tile_pool(name="r_sbuf", bufs=2) as r_pool, \
             tc.tile_pool(name="o_sbuf", bufs=3) as o_pool, \
             tc.tile_pool(name="y_sbuf", bufs=3) as y_pool:
            o_np = {}

            def stage1(b, hp):
                """S^T pair matmul + exp + rpbE multiply -> p_tiles dict."""
                n = b // 2
                q0 = (b % 2) * NTOK
                p_tiles = {}
                for ci, (joff, jlen) in enumerate(TOKC):
                    rpb_sb = rpb0_sb if ci == 0 else rpb1_sb
                    jsl = slice(q0 + joff, q0 + joff + jlen)
                    pss = ps_s.tile([jlen, 2, NTOK], F32, tag=f"s{ci}")
                    nc.tensor.matmul(
                        pss[:, :, :], k_sb[(hp, n)][:, jsl],
                        q_sb[(hp, n)][:, :, q0:q0 + NTOK],
                        start=True, stop=True)
                    praw = praw_pool.tile([jlen, 2, NTOK], BF16,
                                          tag=f"pr{ci}")
                    nc.scalar.activation(praw[:], pss[:, :, :], AF.Exp)
                    pt = p_pool.tile([jlen, 2, NTOK], BF16, tag=f"p{ci}")
                    rpb_pair = rpb_sb[:jlen, 2 * hp:2 * hp + 2, :] \
                        .rearrange("p h n -> p (h n)")
                    eng = nc.gpsimd if ci == 0 else nc.vector
                    eng.tensor_tensor(pt[:].rearrange("p a b -> p (a b)"),
                                      praw[:].rearrange("p a b -> p (a b)"),
                                      rpb_pair, ALU.mult)
                    p_tiles[ci] = pt
                return p_tiles

            def stage2(b, hp, p_tiles):
                """Fused PV+sums, reciprocal, normalize -> o_np tiles.

                lhsT = [ones | v_h]: sums (replicated x64) on partitions
                0:64, O_h^T on 64:128.  One bank per head chain.
                """
                po_h = []
                for h in range(2):
                    po = ps_o.tile([128, 512], F32, tag=f"o{h}",
                                   name=f"po{h}")
                    for ci, (joff, jlen) in enumerate(TOKC):
                        lhsT = v_ci[ci][:, b, hp, h, :, :]
                        nc.tensor.matmul(
                            po[:, 0:NTOK], lhsT, p_tiles[ci][:, h, :],
                            start=(ci == 0), stop=(ci == 1))
                    po_h.append(po)
                rbc = r_pool.tile([64, 2, NTOK], F32, tag="rbc")
                nc.vector.reciprocal_approx_fast(
                    out=rbc[:, 0, :], in_=po_h[0][0:64, 0:NTOK])
                nc.vector.reciprocal_approx_fast(
                    out=rbc[:, 1, :], in_=po_h[1][0:64, 0:NTOK])
                npair = b // 2
                if (npair, hp) not in o_np:
                    o_np[(npair, hp)] = o_pool.tile(
                        [128, 2, NTOK], BF16, tag=f"o{hp}",
                        name=f"on{npair}_{hp}")
                ot = o_np[(npair, hp)]
                side = b % 2
                nc.vector.tensor_tensor(ot[0:64, side, :],
                                        po_h[0][64:128, 0:NTOK],
                                        rbc[:, 0, :], ALU.mult)
                nc.vector.tensor_tensor(ot[64:128, side, :],
                                        po_h[1][64:128, 0:NTOK],
                                        rbc[:, 1, :], ALU.mult)

            def proj_chain(npair, t):
                """One out-proj chain: feats [t*128,(t+1)*128) x 392 toks."""
                psy = ps_o.tile([128, 512], F32, tag=f"o{t % 2}",
                                name=f"psy{npair}_{t}")
                for hp in range(NHP):
                    nc.tensor.matmul(
                        psy[:, 0:2 * NTOK],
                        w_t[3][:, hp, t * 128:(t + 1) * 128],
                        o_np[(npair, hp)][:].rearrange("p a b -> p (a b)"),
                        start=(hp == 0), stop=(hp == NHP - 1))
                yt = y_pool.tile([128, 2 * NTOK], F32, tag="y")
                nc.scalar.activation(yt[:], psy[:, 0:2 * NTOK], AF.Identity,
                                     bias=beff_sb[:, t:t + 1])
                dq = nc.sync if t % 2 == 0 else nc.scalar
                dq.dma_start(
                    y[t, :, npair * 2 * NTOK:(npair + 1) * 2 * NTOK], yt[:])

            blocks = [(b, hp) for b in range(BPC) for hp in range(NHP)]
            pending = {}
            for idx in range(min(STAGGER, len(blocks))):
                pending[idx] = stage1(*blocks[idx])
            for k, (b, hp) in enumerate(blocks):
                nxt = k + STAGGER
                if nxt < len(blocks):
                    pending[nxt] = stage1(*blocks[nxt])
                stage2(b, hp, pending.pop(k))
                # interleave the PREVIOUS npair's proj chains, one per block
                npair, t = (k - 12) // 12, (k - 12) % 12
                if k >= 12 and t < KC:
                    proj_chain(npair, t)
            for t in range(KC):
                proj_chain(BPC // 2 - 1, t)
    nc.finalize()
    return nc


def prep_host(x, qkv_weight, q_bias, v_bias, rpb_table, rel_pos_index,
              proj_weight, proj_bias):
    """Host-side prep: transposes, dtype casts, bias folding, rpb gather."""
    bf16 = ml_dtypes.bfloat16
    x = np.asarray(x, np.float32)
    qkv_weight = np.asarray(qkv_weight, np.float32)
    proj_weight = np.asarray(proj_weight, np.float32)
    q_bias = np.asarray(q_bias, np.float32)
    v_bias = np.asarray(v_bias, np.float32)
    rpb_table = np.asarray(rpb_table, np.float32)
    rel_pos_index = np.asarray(rel_pos_index)
    proj_bias = np.asarray(proj_bias, np.float32)

    # scale folded into q projection weights + bias
    wq = qkv_weight[0:DIM].T * SCALE
    wk = qkv_weight[DIM:2 * DIM].T
    wv = qkv_weight[2 * DIM:3 * DIM].T
    wp = proj_weight.T
    # wq/wk: w_all[p, w, t, kc*128+c] = W^T[kc*128+p, t*128+c] (t-major);
    # wv/wp: w_all[p, w, kc, o] = W^T[kc*128+p, o] (kc-major, as consumed)
    def kc_major(w):
        return w.reshape(KC, 128, DIM).transpose(1, 0, 2)      # [p, kc, o]

    def t_major(w):
        return (w.reshape(KC, 128, KC, 128)                    # [kc, p, t, c]
                .transpose(1, 2, 0, 3).reshape(128, KC, DIM))  # [p, t, kc*c]

    w_all = np.ascontiguousarray(np.stack(
        [t_major(wq), t_major(wk), kc_major(wv), kc_major(wp)],
        axis=1)).astype(bf16)                                  # [p, 4, ., .]
    qb = np.ascontiguousarray((q_bias * SCALE).reshape(KC, 128).T).astype(
        np.float32)
    beff_vec = (proj_bias + proj_weight @ v_bias).astype(np.float32)
    beff = np.ascontiguousarray(beff_vec.reshape(KC, 128).T).astype(np.float32)

    rpb_full = rpb_table[rel_pos_index.reshape(-1)].reshape(NTOK, NTOK, H)
    rpbT = np.exp(np.ascontiguousarray(rpb_full.transpose(1, 2, 0)),
                  dtype=np.float32)
    rpb0 = np.ascontiguousarray(rpbT[0:128]).astype(bf16)
    rpb1 = np.ascontiguousarray(rpbT[128:NTOK]).astype(bf16)

    shared = dict(w_all=w_all, qb=qb, beff=beff, rpb0=rpb0, rpb1=rpb1)
    in_maps = []
    for c in range(N_CORES):
        xs = x[c * BPC:(c + 1) * BPC]                       # [8,196,768]
        x_fm = xs.transpose(2, 0, 1).reshape(DIM, TPC)      # [768, 1568]
        x_tc = np.ascontiguousarray(
            x_fm.reshape(KC, 128, NQCH, NQW).transpose(1, 2, 0, 3)
        ).astype(bf16)
        in_maps.append(dict(shared, x_t=x_tc))
    return in_maps


def unshard_y(y_arr):
    """[KC, 128, TPC] feature-major core output -> [BPC, NTOK, DIM]."""
    return np.ascontiguousarray(
        np.asarray(y_arr).reshape(DIM, TPC).T).reshape(BPC, NTOK, DIM)


_NC_CACHE = {}


def get_nc():
    if "nc" not in _NC_CACHE:
        _NC_CACHE["nc"] = build_nc()
    return _NC_CACHE["nc"]


def kernel(**inputs):
    nc = get_nc()
    in_maps = prep_host(**inputs)
    res = run_bass_kernel_spmd(nc, in_maps, list(range(N_CORES)))
    outs = [unshard_y(res.results[c]["y"]) for c in range(N_CORES)]
    return np.concatenate(outs, axis=0).astype(np.float32)
